# revision 23
# baseline (speedup 1.0000x reference)
"""Trainium2 Bass kernel for nn_Encoder (B=4, S=2048, D=512, H=8 self-attention).

Sharding over 8 NeuronCores: core c -> (batch b = c//2, head-group hg = c%2).
Each core computes, for its batch and its 4 heads, the full attention block
plus a partial output projection y_part = attn_out @ Wo[group rows]. The host
sums the two partial y tensors per batch.

v3: host-side key compaction. The key-padding mask kills ~half the keys
(exp(-1e9) == 0), so the host gathers only the valid key columns of x and
pads to SK = NKT*128 slots (NKT=9 for the ~1044-valid-key regime). Pad
slots are zero columns: their scores are exactly 0, so exp gives exactly
1.0, and they are excluded exactly by (a) V' pad rows being zero (numerator)
and (b) the denominator "ones column" of V' holding the validity mask
instead of all-ones. No attention bias input is needed at all.

The Scalar engine Exp stream is the roofline (~75us for 8 phases). Scores
for (key-tile, head) chunks land in two ping-pong PSUM pools (3 banks +
2 banks) so each ACTIVATE covers N=1536/N=1024 elements, amortizing the
~220-cycle instruction overhead while leaving 3 PSUM banks for the attnV
accumulators ([65, 1024]: dh rows + denominator row, 2 heads) and one for
projection scratch. Score matmuls for the two heads of a pair alternate
row groups 0-63/64-127 so they run concurrently on the PE sub-arrays.
"""

import math

import ml_dtypes
import numpy as np

import concourse.mybir as mybir
import concourse.tile as tile
from concourse import bacc
from concourse.bass_utils import run_bass_kernel_spmd

B, S, D, H = 4, 2048, 512, 8
DH = D // H          # 64
HPC = H // 2         # 4 heads per core
HE = HPC * DH        # 256 output-proj rows per core
T = S
NDC = D // 128       # 4 contraction chunks for projections
NTW = T // 512       # 4 query windows
N_CORES = 8
NKT_DEFAULT = 9      # key tiles (128 keys each) after compaction

f32 = mybir.dt.float32
bf16 = mybir.dt.bfloat16
EXP = mybir.ActivationFunctionType.Exp


def _group_sizes(n_chunks):
    """Split n_chunks score chunks into ACT groups alternating 3/2 wide."""
    sizes = []
    want = 3
    rem = n_chunks
    while rem > 0:
        take = min(want, rem)
        sizes.append(take)
        rem -= take
        want = 2 if want == 3 else 3
    return sizes


def build_nc(nkt=NKT_DEFAULT):
    SK = nkt * 128
    nc = bacc.Bacc("TRN2", target_bir_lowering=False, debug=False, num_devices=1)

    xq = nc.dram_tensor("xq", [D, S], bf16, kind="ExternalInput").ap()
    xk = nc.dram_tensor("xk", [D, SK], bf16, kind="ExternalInput").ap()
    wq = nc.dram_tensor("wq", [D, HE], bf16, kind="ExternalInput").ap()
    wk = nc.dram_tensor("wk", [D, HE], bf16, kind="ExternalInput").ap()
    wv = nc.dram_tensor("wv", [D, HE], bf16, kind="ExternalInput").ap()
    wo = nc.dram_tensor("wo", [HE, D], bf16, kind="ExternalInput").ap()
    vmask = nc.dram_tensor("vmask", [128, nkt * HPC], bf16, kind="ExternalInput").ap()
    y = nc.dram_tensor("y", [T, D], bf16, kind="ExternalOutput").ap()

    # K-piece boundaries for the K' projection (rhs free <= 512)
    kp = []
    off = 0
    while off < SK:
        w = min(512, SK - off)
        kp.append((off, w))
        off += w

    n_chunks = 2 * nkt
    gsizes = _group_sizes(n_chunks)

    with tile.TileContext(nc) as tc:
        with (
            tc.tile_pool(name="const", bufs=1) as const,
            tc.tile_pool(name="psA", bufs=1, space="PSUM") as psA,
            tc.tile_pool(name="psB", bufs=1, space="PSUM") as psB,
            tc.tile_pool(name="psAV", bufs=2, space="PSUM") as psAV,
            tc.tile_pool(name="psM", bufs=1, space="PSUM") as psM,
            tc.tile_pool(name="atp", bufs=8) as at_pool,
            tc.tile_pool(name="yout", bufs=3) as y_pool,
            tc.tile_pool(name="recip", bufs=4) as r_pool,
            tc.tile_pool(name="recipb", bufs=4) as rb_pool,
        ):
            # ---- ACT table warm-up + PE warm-up source tiles ---------------
            warm_i = const.tile([1, 1], f32, tag="warm_i")
            nc.gpsimd.memset(warm_i[:], 0.0)
            ones33 = const.tile([33, 64], bf16, tag="ones33")
            nc.gpsimd.memset(ones33[:], 1.0)
            warm_o = const.tile([1, 1], f32, tag="warm_o")
            nc.scalar.activation(warm_o[:], warm_i[:], EXP)

            # ---- loads (priority order: first-needed first) ----------------
            wk_sb = const.tile([128, NDC, HE], bf16, tag="wk")
            nc.scalar.dma_start(wk_sb[:], wk.rearrange("(c p) n -> p c n", p=128))
            # x staged as SEPARATE tiles per DMA piece so a projection
            # depends only on its own piece's DMA (one shared tile made
            # every reader wait for the LAST piece to land)
            xk_r2 = xk.rearrange("(c p) s -> p c s", p=128)
            xkt = [
                const.tile([128, NDC, w], bf16, tag=f"xk{pi}", name=f"xk{pi}")
                for pi, (off, w) in enumerate(kp)
            ]

            def dma_xk_piece(pi, eng=None):
                off, w = kp[pi]
                (eng or nc.sync).dma_start(
                    xkt[pi][:], xk_r2[:, :, off : off + w]
                )

            dma_xk_piece(0)
            wq_sb = const.tile([128, NDC, HE], bf16, tag="wq")
            nc.scalar.dma_start(wq_sb[:], wq.rearrange("(c p) n -> p c n", p=128))
            xq_r2 = xq.rearrange("(c p) s -> p c s", p=128)
            xqt = [
                const.tile([128, NDC, 512], bf16, tag=f"xq{qw}", name=f"xq{qw}")
                for qw in range(NTW)
            ]

            def dma_xq_piece(qw, eng=None):
                (eng or nc.sync).dma_start(
                    xqt[qw][:], xq_r2[:, :, qw * 512 : (qw + 1) * 512]
                )

            dma_xq_piece(0, eng=nc.gpsimd)
            dma_xk_piece(1)
            wv_sb = const.tile([128, NDC, HE], bf16, tag="wv")
            nc.gpsimd.dma_start(wv_sb[:], wv.rearrange("(c p) n -> p c n", p=128))
            if len(kp) > 2:
                dma_xk_piece(2)

            def xk_cols(vst):
                """(piece tile, col offset) holding key tile vst's columns."""
                base = vst * 128
                for pi, (off, w) in enumerate(kp):
                    if off <= base < off + w:
                        return xkt[pi], base - off
                raise AssertionError(vst)

            # V' tiles: [partition(key in tile)][key-tile][local head][DH + vcol]
            # column DH holds the validity mask (1 valid / 0 pad) so the
            # denominator row of attnV excludes pad keys exactly.
            v_sb = const.tile([128, nkt, HPC, DH + 1], bf16, tag="v")
            vm_sb = const.tile([128, nkt * HPC], bf16, tag="vm")
            nc.sync.dma_start(vm_sb[:], vmask[:, :])
            nc.vector.tensor_copy(
                v_sb[:, :, :, DH],
                vm_sb[:].rearrange("p (j h) -> p j h", h=HPC),
            )
            dma_xq_piece(1)
            dma_xq_piece(2)
            dma_xq_piece(3)
            wo_sb = const.tile([128, HE // 128, D], bf16, tag="wo")
            nc.sync.dma_start(wo_sb[:], wo.rearrange("(c p) n -> p c n", p=128))

            kt = [
                const.tile([128, SK], bf16, tag=f"kt{pp}", name=f"kt{pp}")
                for pp in range(2)
            ]
            qt = [
                const.tile([128, S], bf16, tag=f"qt{pp}", name=f"qt{pp}")
                for pp in range(2)
            ]
            # outT [he, t] as [128, 2, T]: chunk pp, rows h2*64
            outT_sb = const.tile([128, HE // 128, T], bf16, tag="outT")

            # PE keep-warm: tiny matmuls with no data deps run during the
            # DMA window so the HAM clock gate stays open.
            def emit_pe_warm(n):
                warm_ps = psAV.tile([64, 64], f32, tag="av", name="warm_ps")
                for _ in range(n):
                    nc.tensor.matmul(
                        warm_ps[:],
                        lhsT=ones33[0:1, :],
                        rhs=ones33[0:1, :],
                        start=True,
                        stop=True,
                    )

            emit_pe_warm(12)

            # ---- background work units -------------------------------------
            kq_done: set[tuple] = set()
            v_done = [False] * nkt

            def emit_k_piece(pp, pi, pool=None):
                key = ("k", pp, pi)
                if key in kq_done:
                    return
                kq_done.add(key)
                off, w = kp[pi]
                ps = (pool or psM).tile(
                    [128, 512], f32, tag="mm" if pool is None else "sc", name="k_ps"
                )
                for dc in range(NDC):
                    nc.tensor.matmul(
                        ps[:, 0:w],
                        lhsT=wk_sb[:, dc, pp * 128 : (pp + 1) * 128],
                        rhs=xkt[pi][:, dc, :],
                        start=(dc == 0),
                        stop=(dc == NDC - 1),
                    )
                nc.vector.tensor_copy(kt[pp][:, off : off + w], ps[:, 0:w])

            def emit_q_piece(pp, qw, pool=None):
                key = ("q", pp, qw)
                if key in kq_done:
                    return
                kq_done.add(key)
                ps = (pool or psM).tile(
                    [128, 512], f32, tag="mm" if pool is None else "sc", name="q_ps"
                )
                for dc in range(NDC):
                    nc.tensor.matmul(
                        ps[:],
                        lhsT=wq_sb[:, dc, pp * 128 : (pp + 1) * 128],
                        rhs=xqt[qw][:, dc, :],
                        start=(dc == 0),
                        stop=(dc == NDC - 1),
                    )
                nc.vector.tensor_copy(qt[pp][:, qw * 512 : (qw + 1) * 512], ps[:])

            def emit_v_chain(vst, pool=None):
                if v_done[vst]:
                    return
                v_done[vst] = True
                ps = (pool or psM).tile(
                    [128, HE], f32, tag="mm" if pool is None else "sc", name="v_ps"
                )
                xk_t, coff = xk_cols(vst)
                for dc in range(NDC):
                    nc.tensor.matmul(
                        ps[:],
                        lhsT=xk_t[:, dc, coff : coff + 128],
                        rhs=wv_sb[:, dc, :],
                        start=(dc == 0),
                        stop=(dc == NDC - 1),
                    )
                nc.vector.tensor_copy(
                    v_sb[:, vst, :, 0:DH],
                    ps[:].rearrange("p (h e) -> p h e", e=DH),
                )

            def emit_wo_tt(tt, pool=None, eng=None):
                ps = (pool or psM).tile(
                    [128, 512], f32, tag="mm" if pool is None else "sc", name="y_ps"
                )
                for c in range(HE // 128):
                    nc.tensor.matmul(
                        ps[:],
                        lhsT=outT_sb[:, c, tt * 128 : (tt + 1) * 128],
                        rhs=wo_sb[:, c, :],
                        start=(c == 0),
                        stop=(c == HE // 128 - 1),
                    )
                y_sb = y_pool.tile([128, 512], bf16, tag="y", name="y_sb")
                nc.vector.tensor_copy(y_sb[:], ps[:])
                (eng or nc.gpsimd).dma_start(y[tt * 128 : (tt + 1) * 128, :], y_sb[:])

            def run_unit(u):
                if u[0] == "v":
                    emit_v_chain(u[1])
                elif u[0] == "k":
                    emit_k_piece(u[1], u[2])
                elif u[0] == "q":
                    emit_q_piece(u[1], u[2])
                else:
                    # a Wo unit reads outT for its query window: both of
                    # that window's phases must have been normalized (i.e.
                    # fully drained) BEFORE this emission, else the matmul
                    # reads stale outT (program order is the data)
                    req = (u[1] // 4) * 2 + 1
                    while drained[req] < CPP:
                        if not drain_one():
                            break
                    emit_wo_tt(u[1])

            # phases: qw-major, pp-inner so Wo(qw) unblocks early
            phases = [(qw, pp) for qw in range(NTW) for pp in range(2)]
            NPH = len(phases)
            CPP = 2 * nkt          # chunks per phase
            NCH = NPH * CPP        # global chunk count

            # Global ACT-group stream: groups alternate between the 3-bank
            # (A) and 2-bank (B) PSUM pools and may SPAN phase boundaries
            # (the Exp scale is uniform), so the A/B ping-pong never stalls
            # at a phase transition.
            gsz = []
            rem, cap = NCH, 3
            while rem > 0:
                gsz.append(min(cap, rem))
                rem -= gsz[-1]
                cap = 2 if cap == 3 else 3

            def chunk_info(C):
                p, c = C // CPP, C % CPP
                return p, c // 2, c % 2   # phase, key tile, head-in-pair

            pend: list[tuple] = []       # (at_tile, [global chunk ids])
            av_by_phase: dict[int, list] = {}
            drained = [0] * NPH

            def emit_scores_group(C0, size, pool):
                width = size * 512
                sc = pool.tile([128, width], f32, tag="sc", name="sc")
                with tc.high_priority(offset=40):
                    for i in range(size):
                        p, ktile, h2 = chunk_info(C0 + i)
                        qw, pp = phases[p]
                        nc.tensor.matmul(
                            sc[:, i * 512 : (i + 1) * 512],
                            lhsT=kt[pp][
                                h2 * 64 : (h2 + 1) * 64,
                                ktile * 128 : (ktile + 1) * 128,
                            ],
                            rhs=qt[pp][
                                h2 * 64 : (h2 + 1) * 64, qw * 512 : (qw + 1) * 512
                            ],
                            start=True,
                            stop=True,
                        )
                at_t = at_pool.tile([128, width], bf16, tag="at", name="at")
                nc.scalar.activation(at_t[:], sc[:], EXP, scale=0.125)
                pend.append((at_t, list(range(C0, C0 + size))))

            def emit_normalize(p):
                qw_, pp_ = phases[p]
                av_ = av_by_phase[p]
                for h2 in range(2):
                    # denominator row must be copied to a partition-0 SBUF
                    # tile first: the custom-DVE reciprocal cannot read the
                    # PSUM row at partition offset 64 directly
                    rt = r_pool.tile([1, 512], f32, tag="rt", name="rt")
                    nc.vector.tensor_copy(rt[0:1, :], av_[h2][DH : DH + 1, :])
                    ri = r_pool.tile([1, 512], f32, tag="ri", name="ri")
                    nc.vector.reciprocal_approx_fast(ri[0:1, :], rt[0:1, :])
                    rb = rb_pool.tile([64, 512], f32, tag="rb", name="rb")
                    nc.gpsimd.partition_broadcast(rb[:], ri[0:1, :])
                    nc.vector.tensor_mul(
                        outT_sb[
                            h2 * 64 : (h2 + 1) * 64,
                            pp_,
                            qw_ * 512 : (qw_ + 1) * 512,
                        ],
                        av_[h2][0:DH, :],
                        rb[:],
                    )

            def drain_one():
                if not pend:
                    return False
                at_t, chunks = pend.pop(0)
                for C in chunks:
                    p_, ktile, h2 = chunk_info(C)
                    if not v_done[ktile]:
                        emit_v_chain(ktile)
                for i, C in enumerate(chunks):
                    p_, ktile, h2 = chunk_info(C)
                    pp_ = phases[p_][1]
                    if p_ not in av_by_phase:
                        av_by_phase[p_] = [
                            psAV.tile([DH + 1, 512], f32, tag="av", name=f"av{h}")
                            for h in range(2)
                        ]
                    nc.tensor.matmul(
                        av_by_phase[p_][h2][:],
                        lhsT=v_sb[:, ktile, 2 * pp_ + h2, :],
                        rhs=at_t[:, i * 512 : (i + 1) * 512],
                        start=(ktile == 0),
                        stop=(ktile == nkt - 1),
                    )
                    drained[p_] += 1
                    if drained[p_] == CPP:
                        emit_normalize(p_)
                return True

            # background units, scheduled per phase (hard deps enforced by
            # Tile; ordering shapes engine pacing and respects DMA arrival;
            # every K'/V'/Q' must be EMITTED before its first reader since
            # Tile does not reorder a read ahead of a later write).
            ph0 = [("q", 1, 0)]
            if len(kp) > 1:
                ph0.append(("k", 0, 1))
            ph0 += [("v", 0), ("v", 1)]
            if len(kp) > 2:
                ph0.append(("k", 0, 2))
            ph0 += [("k", 1, 0), ("v", 2)]
            if len(kp) > 1:
                ph0.append(("k", 1, 1))
            ph0.append(("v", 3))
            if len(kp) > 2:
                ph0.append(("k", 1, 2))
            ph0 += [("v", i) for i in range(4, nkt)]
            bg_by_phase = {
                0: ph0,
                1: [("q", 0, 1)],
                2: [("q", 1, 1), ("wo", 0), ("wo", 1)],
                3: [("q", 0, 2), ("wo", 2), ("wo", 3)],
                4: [("q", 1, 2), ("wo", 4), ("wo", 5)],
                5: [("q", 0, 3), ("wo", 6), ("wo", 7)],
                6: [("q", 1, 3), ("wo", 8), ("wo", 9)],
                7: [("wo", 10), ("wo", 11)],
            }

            # ---- prologue: minimum inputs for the first groups -------------
            emit_k_piece(0, 0, pool=psA)
            emit_q_piece(0, 0, pool=psB)

            # ---- main global group loop ------------------------------------
            units: list[tuple] = []
            seen_phase = -1
            C0 = 0
            for gi, size in enumerate(gsz):
                pool = psA if size == 3 else psB
                p_first = chunk_info(C0)[0]
                if p_first > seen_phase:
                    for p in range(seen_phase + 1, p_first + 1):
                        units.extend(bg_by_phase.get(p, []))
                    seen_phase = p_first
                emit_scores_group(C0, size, pool)
                C0 += size
                n_units = 2 if len(units) > 7 else 1
                for _ in range(n_units):
                    if units:
                        run_unit(units.pop(0))
                while len(pend) > 1:
                    if not drain_one():
                        break

            # ---- tail: drain, normalize via drained-trigger, last Wo -------
            while drain_one():
                pass
            for u in units:
                run_unit(u)
            emit_wo_tt(12, pool=psA, eng=nc.sync)
            emit_wo_tt(13, pool=psB, eng=nc.sync)
            emit_wo_tt(14, eng=nc.sync)
            emit_wo_tt(15, pool=psA, eng=nc.sync)

    nc.compile()
    return nc


_NC_CACHE: dict[int, object] = {}


def _get_nc(nkt=NKT_DEFAULT):
    if nkt not in _NC_CACHE:
        _NC_CACHE[nkt] = build_nc(nkt)
    return _NC_CACHE[nkt]


def make_in_maps(x, mask, Wq, Wk, Wv, Wo, nkt=None):
    bf = ml_dtypes.bfloat16
    mask = np.asarray(mask)
    counts = (mask > 0).sum(axis=1)
    if nkt is None:
        nkt = max(1, int(math.ceil(counts.max() / 128)))
    SK = nkt * 128

    xqT = np.ascontiguousarray(x.transpose(0, 2, 1)).astype(bf)  # [B, D, S]
    # [H, D, DH] -> [D, H*DH]
    wq_f = np.ascontiguousarray(Wq.transpose(1, 0, 2).reshape(D, H * DH))
    wk_f = np.ascontiguousarray(Wk.transpose(1, 0, 2).reshape(D, H * DH))
    wv_f = np.ascontiguousarray(Wv.transpose(1, 0, 2).reshape(D, H * DH))

    xkT = []
    vmasks = []
    for b in range(B):
        idx = np.flatnonzero(mask[b] > 0)
        nv = len(idx)
        xk_b = np.zeros((SK, D), np.float32)
        xk_b[:nv] = x[b][idx]
        xkT.append(np.ascontiguousarray(xk_b.T).astype(bf))
        vm = np.zeros((128, nkt, HPC), np.float32)
        slot = np.arange(nkt * 128).reshape(nkt, 128)
        vm[:, :, :] = (slot.T[:, :, None] < nv).astype(np.float32)
        vmasks.append(vm.reshape(128, nkt * HPC).astype(bf))

    in_maps = []
    for c in range(N_CORES):
        b, hg = c // 2, c % 2
        cols = slice(hg * HE, (hg + 1) * HE)
        in_maps.append(
            {
                "xq": xqT[b],
                "xk": xkT[b],
                "wq": np.ascontiguousarray(wq_f[:, cols]).astype(bf),
                "wk": np.ascontiguousarray(wk_f[:, cols]).astype(bf),
                "wv": np.ascontiguousarray(wv_f[:, cols]).astype(bf),
                "wo": np.ascontiguousarray(Wo[cols, :]).astype(bf),
                "vmask": vmasks[b],
            }
        )
    return in_maps, nkt


def combine_results(results):
    y = np.zeros((B, S, D), np.float32)
    for c in range(N_CORES):
        y[c // 2] += results[c]["y"].astype(np.float32)
    return y


def kernel(x, mask, Wq, Wk, Wv, Wo):
    in_maps, nkt = make_in_maps(
        np.asarray(x, np.float32),
        np.asarray(mask),
        np.asarray(Wq, np.float32),
        np.asarray(Wk, np.float32),
        np.asarray(Wv, np.float32),
        np.asarray(Wo, np.float32),
    )
    nc = _get_nc(nkt)
    res = run_bass_kernel_spmd(nc, in_maps, core_ids=list(range(N_CORES)))
    return combine_results(res.results)


# revision 24
# speedup vs baseline: 1.0087x; 1.0087x over previous
"""Trainium2 Bass kernel for nn_Encoder (B=4, S=2048, D=512, H=8 self-attention).

Sharding over 8 NeuronCores: core c -> (batch b = c//2, head-group hg = c%2).
Each core computes, for its batch and its 4 heads, the full attention block
plus a partial output projection y_part = attn_out @ Wo[group rows]. The host
sums the two partial y tensors per batch.

v3: host-side key compaction. The key-padding mask kills ~half the keys
(exp(-1e9) == 0), so the host gathers only the valid key columns of x and
pads to SK = NKT*128 slots (NKT=9 for the ~1044-valid-key regime). Pad
slots are zero columns: their scores are exactly 0, so exp gives exactly
1.0, and they are excluded exactly by (a) V' pad rows being zero (numerator)
and (b) the denominator "ones column" of V' holding the validity mask
instead of all-ones. No attention bias input is needed at all.

The Scalar engine Exp stream is the roofline (~75us for 8 phases). Scores
for (key-tile, head) chunks land in two ping-pong PSUM pools (3 banks +
2 banks) so each ACTIVATE covers N=1536/N=1024 elements, amortizing the
~220-cycle instruction overhead while leaving 3 PSUM banks for the attnV
accumulators ([65, 1024]: dh rows + denominator row, 2 heads) and one for
projection scratch. Score matmuls for the two heads of a pair alternate
row groups 0-63/64-127 so they run concurrently on the PE sub-arrays.
"""

import math

import ml_dtypes
import numpy as np

import concourse.mybir as mybir
import concourse.tile as tile
from concourse import bacc
from concourse.bass_utils import run_bass_kernel_spmd

B, S, D, H = 4, 2048, 512, 8
DH = D // H          # 64
HPC = H // 2         # 4 heads per core
HE = HPC * DH        # 256 output-proj rows per core
T = S
NDC = D // 128       # 4 contraction chunks for projections
NTW = T // 512       # 4 query windows
N_CORES = 8
NKT_DEFAULT = 9      # key tiles (128 keys each) after compaction

f32 = mybir.dt.float32
bf16 = mybir.dt.bfloat16
EXP = mybir.ActivationFunctionType.Exp


def _group_sizes(n_chunks):
    """Split n_chunks score chunks into ACT groups alternating 3/2 wide."""
    sizes = []
    want = 3
    rem = n_chunks
    while rem > 0:
        take = min(want, rem)
        sizes.append(take)
        rem -= take
        want = 2 if want == 3 else 3
    return sizes


def build_nc(nkt=NKT_DEFAULT):
    SK = nkt * 128
    nc = bacc.Bacc("TRN2", target_bir_lowering=False, debug=False, num_devices=1)

    xq = nc.dram_tensor("xq", [D, S], bf16, kind="ExternalInput").ap()
    xk = nc.dram_tensor("xk", [D, SK], bf16, kind="ExternalInput").ap()
    wq = nc.dram_tensor("wq", [128, NDC * HE], bf16, kind="ExternalInput").ap()
    wk = nc.dram_tensor("wk", [128, NDC * HE], bf16, kind="ExternalInput").ap()
    wv = nc.dram_tensor("wv", [128, NDC * HE], bf16, kind="ExternalInput").ap()
    wo = nc.dram_tensor("wo", [128, (HE // 128) * D], bf16, kind="ExternalInput").ap()
    vmask = nc.dram_tensor("vmask", [128, nkt * HPC], bf16, kind="ExternalInput").ap()
    y = nc.dram_tensor("y", [T, D], bf16, kind="ExternalOutput").ap()

    # K-piece boundaries for the K' projection (rhs free <= 512)
    kp = []
    off = 0
    while off < SK:
        w = min(512, SK - off)
        kp.append((off, w))
        off += w

    n_chunks = 2 * nkt
    gsizes = _group_sizes(n_chunks)

    with tile.TileContext(nc) as tc:
        with (
            tc.tile_pool(name="const", bufs=1) as const,
            tc.tile_pool(name="psA", bufs=1, space="PSUM") as psA,
            tc.tile_pool(name="psB", bufs=1, space="PSUM") as psB,
            tc.tile_pool(name="psAV", bufs=2, space="PSUM") as psAV,
            tc.tile_pool(name="psM", bufs=1, space="PSUM") as psM,
            tc.tile_pool(name="atp", bufs=8) as at_pool,
            tc.tile_pool(name="yout", bufs=3) as y_pool,
            tc.tile_pool(name="recip", bufs=4) as r_pool,
            tc.tile_pool(name="recipb", bufs=4) as rb_pool,
        ):
            # ---- ACT table warm-up + PE warm-up source tiles ---------------
            warm_i = const.tile([1, 1], f32, tag="warm_i")
            nc.gpsimd.memset(warm_i[:], 0.0)
            ones33 = const.tile([33, 64], bf16, tag="ones33")
            nc.gpsimd.memset(ones33[:], 1.0)
            warm_o = const.tile([1, 1], f32, tag="warm_o")
            nc.scalar.activation(warm_o[:], warm_i[:], EXP)

            # ---- loads (priority order: first-needed first) ----------------
            wk_sb = const.tile([128, NDC, HE], bf16, tag="wk")
            wk_r = wk.rearrange("p (c n) -> p c n", c=NDC)
            nc.scalar.dma_start(wk_sb[:, 0:2, :], wk_r[:, 0:2, :])
            nc.scalar.dma_start(wk_sb[:, 2:4, :], wk_r[:, 2:4, :])
            # x staged as SEPARATE tiles per DMA piece so a projection
            # depends only on its own piece's DMA (one shared tile made
            # every reader wait for the LAST piece to land)
            xk_r2 = xk.rearrange("(c p) s -> p c s", p=128)
            xkt = [
                const.tile([128, NDC, w], bf16, tag=f"xk{pi}", name=f"xk{pi}")
                for pi, (off, w) in enumerate(kp)
            ]

            def dma_xk_piece(pi, eng=None, split=False):
                off, w = kp[pi]
                if split:
                    for c in range(NDC):
                        (eng or nc.sync).dma_start(
                            xkt[pi][:, c, :], xk_r2[:, c, off : off + w]
                        )
                else:
                    (eng or nc.sync).dma_start(
                        xkt[pi][:], xk_r2[:, :, off : off + w]
                    )

            dma_xk_piece(0, split=True)
            wq_sb = const.tile([128, NDC, HE], bf16, tag="wq")
            wq_r = wq.rearrange("p (c n) -> p c n", c=NDC)
            nc.scalar.dma_start(wq_sb[:, 0:2, :], wq_r[:, 0:2, :])
            nc.scalar.dma_start(wq_sb[:, 2:4, :], wq_r[:, 2:4, :])
            xq_r2 = xq.rearrange("(c p) s -> p c s", p=128)
            xqt = [
                const.tile([128, NDC, 512], bf16, tag=f"xq{qw}", name=f"xq{qw}")
                for qw in range(NTW)
            ]

            def dma_xq_piece(qw, eng=None, split=False):
                if split:
                    for c in range(NDC):
                        (eng or nc.sync).dma_start(
                            xqt[qw][:, c, :],
                            xq_r2[:, c, qw * 512 : (qw + 1) * 512],
                        )
                else:
                    (eng or nc.sync).dma_start(
                        xqt[qw][:], xq_r2[:, :, qw * 512 : (qw + 1) * 512]
                    )

            dma_xq_piece(0, eng=nc.gpsimd, split=True)
            dma_xk_piece(1)
            wv_sb = const.tile([128, NDC, HE], bf16, tag="wv")
            nc.gpsimd.dma_start(wv_sb[:], wv.rearrange("p (c n) -> p c n", c=NDC))
            if len(kp) > 2:
                dma_xk_piece(2)

            def xk_cols(vst):
                """(piece tile, col offset) holding key tile vst's columns."""
                base = vst * 128
                for pi, (off, w) in enumerate(kp):
                    if off <= base < off + w:
                        return xkt[pi], base - off
                raise AssertionError(vst)

            # V' tiles: [partition(key in tile)][key-tile][local head][DH + vcol]
            # column DH holds the validity mask (1 valid / 0 pad) so the
            # denominator row of attnV excludes pad keys exactly.
            v_sb = const.tile([128, nkt, HPC, DH + 1], bf16, tag="v")
            vm_sb = const.tile([128, nkt * HPC], bf16, tag="vm")
            nc.sync.dma_start(vm_sb[:], vmask[:, :])
            nc.vector.tensor_copy(
                v_sb[:, :, :, DH],
                vm_sb[:].rearrange("p (j h) -> p j h", h=HPC),
            )
            dma_xq_piece(1)
            dma_xq_piece(2)
            dma_xq_piece(3)
            wo_sb = const.tile([128, HE // 128, D], bf16, tag="wo")
            nc.sync.dma_start(wo_sb[:], wo.rearrange("p (c n) -> p c n", c=HE // 128))

            kt = [
                const.tile([128, SK], bf16, tag=f"kt{pp}", name=f"kt{pp}")
                for pp in range(2)
            ]
            qt = [
                const.tile([128, S], bf16, tag=f"qt{pp}", name=f"qt{pp}")
                for pp in range(2)
            ]
            # outT [he, t] as [128, 2, T]: chunk pp, rows h2*64
            outT_sb = const.tile([128, HE // 128, T], bf16, tag="outT")

            # PE keep-warm: tiny matmuls with no data deps run during the
            # DMA window so the HAM clock gate stays open.
            def emit_pe_warm(n):
                warm_ps = psAV.tile([64, 64], f32, tag="av", name="warm_ps")
                for _ in range(n):
                    nc.tensor.matmul(
                        warm_ps[:],
                        lhsT=ones33[0:1, :],
                        rhs=ones33[0:1, :],
                        start=True,
                        stop=True,
                    )

            emit_pe_warm(12)

            # ---- background work units -------------------------------------
            kq_done: set[tuple] = set()
            v_done = [False] * nkt

            def emit_k_piece(pp, pi, pool=None):
                key = ("k", pp, pi)
                if key in kq_done:
                    return
                kq_done.add(key)
                off, w = kp[pi]
                ps = (pool or psM).tile(
                    [128, 512], f32, tag="mm" if pool is None else "sc", name="k_ps"
                )
                for dc in range(NDC):
                    nc.tensor.matmul(
                        ps[:, 0:w],
                        lhsT=wk_sb[:, dc, pp * 128 : (pp + 1) * 128],
                        rhs=xkt[pi][:, dc, :],
                        start=(dc == 0),
                        stop=(dc == NDC - 1),
                    )
                nc.vector.tensor_copy(kt[pp][:, off : off + w], ps[:, 0:w])

            def emit_q_piece(pp, qw, pool=None):
                key = ("q", pp, qw)
                if key in kq_done:
                    return
                kq_done.add(key)
                ps = (pool or psM).tile(
                    [128, 512], f32, tag="mm" if pool is None else "sc", name="q_ps"
                )
                for dc in range(NDC):
                    nc.tensor.matmul(
                        ps[:],
                        lhsT=wq_sb[:, dc, pp * 128 : (pp + 1) * 128],
                        rhs=xqt[qw][:, dc, :],
                        start=(dc == 0),
                        stop=(dc == NDC - 1),
                    )
                nc.vector.tensor_copy(qt[pp][:, qw * 512 : (qw + 1) * 512], ps[:])

            def emit_v_chain(vst, pool=None):
                if v_done[vst]:
                    return
                v_done[vst] = True
                ps = (pool or psM).tile(
                    [128, HE], f32, tag="mm" if pool is None else "sc", name="v_ps"
                )
                xk_t, coff = xk_cols(vst)
                for dc in range(NDC):
                    nc.tensor.matmul(
                        ps[:],
                        lhsT=xk_t[:, dc, coff : coff + 128],
                        rhs=wv_sb[:, dc, :],
                        start=(dc == 0),
                        stop=(dc == NDC - 1),
                    )
                nc.vector.tensor_copy(
                    v_sb[:, vst, :, 0:DH],
                    ps[:].rearrange("p (h e) -> p h e", e=DH),
                )

            def emit_wo_tt(tt, pool=None, eng=None):
                ps = (pool or psM).tile(
                    [128, 512], f32, tag="mm" if pool is None else "sc", name="y_ps"
                )
                for c in range(HE // 128):
                    nc.tensor.matmul(
                        ps[:],
                        lhsT=outT_sb[:, c, tt * 128 : (tt + 1) * 128],
                        rhs=wo_sb[:, c, :],
                        start=(c == 0),
                        stop=(c == HE // 128 - 1),
                    )
                y_sb = y_pool.tile([128, 512], bf16, tag="y", name="y_sb")
                nc.vector.tensor_copy(y_sb[:], ps[:])
                (eng or nc.gpsimd).dma_start(y[tt * 128 : (tt + 1) * 128, :], y_sb[:])

            def run_unit(u):
                if u[0] == "v":
                    emit_v_chain(u[1])
                elif u[0] == "k":
                    emit_k_piece(u[1], u[2])
                elif u[0] == "q":
                    emit_q_piece(u[1], u[2])
                else:
                    # a Wo unit reads outT for its query window: both of
                    # that window's phases must have been normalized (i.e.
                    # fully drained) BEFORE this emission, else the matmul
                    # reads stale outT (program order is the data)
                    req = (u[1] // 4) * 2 + 1
                    while drained[req] < CPP:
                        if not drain_one():
                            break
                    emit_wo_tt(u[1])

            # phases: qw-major, pp-inner so Wo(qw) unblocks early
            phases = [(qw, pp) for qw in range(NTW) for pp in range(2)]
            NPH = len(phases)
            CPP = 2 * nkt          # chunks per phase
            NCH = NPH * CPP        # global chunk count

            # Global ACT-group stream: groups alternate between the 3-bank
            # (A) and 2-bank (B) PSUM pools and may SPAN phase boundaries
            # (the Exp scale is uniform), so the A/B ping-pong never stalls
            # at a phase transition.
            gsz = []
            rem, cap = NCH, 3
            while rem > 0:
                gsz.append(min(cap, rem))
                rem -= gsz[-1]
                cap = 2 if cap == 3 else 3

            def chunk_info(C):
                p, c = C // CPP, C % CPP
                return p, c // 2, c % 2   # phase, key tile, head-in-pair

            pend: list[tuple] = []       # (at_tile, [global chunk ids])
            av_by_phase: dict[int, list] = {}
            drained = [0] * NPH

            def emit_scores_group(C0, size, pool):
                width = size * 512
                sc = pool.tile([128, width], f32, tag="sc", name="sc")
                with tc.high_priority(offset=40):
                    for i in range(size):
                        p, ktile, h2 = chunk_info(C0 + i)
                        qw, pp = phases[p]
                        nc.tensor.matmul(
                            sc[:, i * 512 : (i + 1) * 512],
                            lhsT=kt[pp][
                                h2 * 64 : (h2 + 1) * 64,
                                ktile * 128 : (ktile + 1) * 128,
                            ],
                            rhs=qt[pp][
                                h2 * 64 : (h2 + 1) * 64, qw * 512 : (qw + 1) * 512
                            ],
                            start=True,
                            stop=True,
                        )
                at_t = at_pool.tile([128, width], bf16, tag="at", name="at")
                nc.scalar.activation(at_t[:], sc[:], EXP, scale=0.125)
                pend.append((at_t, list(range(C0, C0 + size))))

            def emit_normalize(p):
                qw_, pp_ = phases[p]
                av_ = av_by_phase[p]
                for h2 in range(2):
                    # denominator row must be copied to a partition-0 SBUF
                    # tile first: the custom-DVE reciprocal cannot read the
                    # PSUM row at partition offset 64 directly
                    rt = r_pool.tile([1, 512], f32, tag="rt", name="rt")
                    nc.vector.tensor_copy(rt[0:1, :], av_[h2][DH : DH + 1, :])
                    ri = r_pool.tile([1, 512], f32, tag="ri", name="ri")
                    nc.vector.reciprocal_approx_fast(ri[0:1, :], rt[0:1, :])
                    rb = rb_pool.tile([64, 512], f32, tag="rb", name="rb")
                    nc.gpsimd.partition_broadcast(rb[:], ri[0:1, :])
                    nc.vector.tensor_mul(
                        outT_sb[
                            h2 * 64 : (h2 + 1) * 64,
                            pp_,
                            qw_ * 512 : (qw_ + 1) * 512,
                        ],
                        av_[h2][0:DH, :],
                        rb[:],
                    )

            def drain_one():
                if not pend:
                    return False
                at_t, chunks = pend.pop(0)
                for C in chunks:
                    p_, ktile, h2 = chunk_info(C)
                    if not v_done[ktile]:
                        emit_v_chain(ktile)
                for i, C in enumerate(chunks):
                    p_, ktile, h2 = chunk_info(C)
                    pp_ = phases[p_][1]
                    if p_ not in av_by_phase:
                        av_by_phase[p_] = [
                            psAV.tile([DH + 1, 512], f32, tag="av", name=f"av{h}")
                            for h in range(2)
                        ]
                    nc.tensor.matmul(
                        av_by_phase[p_][h2][:],
                        lhsT=v_sb[:, ktile, 2 * pp_ + h2, :],
                        rhs=at_t[:, i * 512 : (i + 1) * 512],
                        start=(ktile == 0),
                        stop=(ktile == nkt - 1),
                    )
                    drained[p_] += 1
                    if drained[p_] == CPP:
                        emit_normalize(p_)
                return True

            # background units, scheduled per phase (hard deps enforced by
            # Tile; ordering shapes engine pacing and respects DMA arrival;
            # every K'/V'/Q' must be EMITTED before its first reader since
            # Tile does not reorder a read ahead of a later write).
            ph0 = [("q", 1, 0)]
            if len(kp) > 1:
                ph0.append(("k", 0, 1))
            ph0 += [("v", 0), ("v", 1)]
            if len(kp) > 2:
                ph0.append(("k", 0, 2))
            ph0 += [("k", 1, 0), ("v", 2)]
            if len(kp) > 1:
                ph0.append(("k", 1, 1))
            ph0.append(("v", 3))
            if len(kp) > 2:
                ph0.append(("k", 1, 2))
            ph0 += [("v", i) for i in range(4, nkt)]
            bg_by_phase = {
                0: ph0,
                1: [("q", 0, 1)],
                2: [("q", 1, 1), ("wo", 0), ("wo", 1)],
                3: [("q", 0, 2), ("wo", 2), ("wo", 3)],
                4: [("q", 1, 2), ("wo", 4), ("wo", 5)],
                5: [("q", 0, 3), ("wo", 6), ("wo", 7)],
                6: [("q", 1, 3), ("wo", 8), ("wo", 9)],
                7: [("wo", 10), ("wo", 11)],
            }

            # ---- prologue: minimum inputs for the first groups -------------
            emit_k_piece(0, 0, pool=psA)
            emit_q_piece(0, 0, pool=psB)

            # ---- main global group loop ------------------------------------
            units: list[tuple] = []
            seen_phase = -1
            C0 = 0
            for gi, size in enumerate(gsz):
                pool = psA if size == 3 else psB
                p_first = chunk_info(C0)[0]
                if p_first > seen_phase:
                    for p in range(seen_phase + 1, p_first + 1):
                        units.extend(bg_by_phase.get(p, []))
                    seen_phase = p_first
                emit_scores_group(C0, size, pool)
                C0 += size
                n_units = 2 if len(units) > 7 else 1
                for _ in range(n_units):
                    if units:
                        run_unit(units.pop(0))
                while len(pend) > 1:
                    if not drain_one():
                        break

            # ---- tail: drain, normalize via drained-trigger, last Wo -------
            while drain_one():
                pass
            for u in units:
                run_unit(u)
            emit_wo_tt(12, pool=psA, eng=nc.sync)
            emit_wo_tt(13, pool=psB, eng=nc.sync)
            emit_wo_tt(14, eng=nc.sync)
            emit_wo_tt(15, pool=psA, eng=nc.sync)

    nc.compile()
    return nc


_NC_CACHE: dict[int, object] = {}


def _get_nc(nkt=NKT_DEFAULT):
    if nkt not in _NC_CACHE:
        _NC_CACHE[nkt] = build_nc(nkt)
    return _NC_CACHE[nkt]


def make_in_maps(x, mask, Wq, Wk, Wv, Wo, nkt=None):
    bf = ml_dtypes.bfloat16
    mask = np.asarray(mask)
    counts = (mask > 0).sum(axis=1)
    if nkt is None:
        nkt = max(1, int(math.ceil(counts.max() / 128)))
    SK = nkt * 128

    xqT = np.ascontiguousarray(x.transpose(0, 2, 1)).astype(bf)  # [B, D, S]
    # [H, D, DH] -> [D, H*DH]
    wq_f = np.ascontiguousarray(Wq.transpose(1, 0, 2).reshape(D, H * DH))
    wk_f = np.ascontiguousarray(Wk.transpose(1, 0, 2).reshape(D, H * DH))
    wv_f = np.ascontiguousarray(Wv.transpose(1, 0, 2).reshape(D, H * DH))

    xkT = []
    vmasks = []
    for b in range(B):
        idx = np.flatnonzero(mask[b] > 0)
        nv = len(idx)
        xk_b = np.zeros((SK, D), np.float32)
        xk_b[:nv] = x[b][idx]
        xkT.append(np.ascontiguousarray(xk_b.T).astype(bf))
        vm = np.zeros((128, nkt, HPC), np.float32)
        slot = np.arange(nkt * 128).reshape(nkt, 128)
        vm[:, :, :] = (slot.T[:, :, None] < nv).astype(np.float32)
        vmasks.append(vm.reshape(128, nkt * HPC).astype(bf))

    def tile_w(w):
        # [D, HE] -> [128, NDC*HE]: sbuf-resident layout, contiguous DMA
        return np.ascontiguousarray(
            w.reshape(NDC, 128, HE).transpose(1, 0, 2).reshape(128, NDC * HE)
        ).astype(bf)

    def tile_wo(w):
        # [HE, D] -> [128, (HE//128)*D]
        return np.ascontiguousarray(
            w.reshape(HE // 128, 128, D).transpose(1, 0, 2).reshape(128, -1)
        ).astype(bf)

    in_maps = []
    for c in range(N_CORES):
        b, hg = c // 2, c % 2
        cols = slice(hg * HE, (hg + 1) * HE)
        in_maps.append(
            {
                "xq": xqT[b],
                "xk": xkT[b],
                "wq": tile_w(wq_f[:, cols]),
                "wk": tile_w(wk_f[:, cols]),
                "wv": tile_w(wv_f[:, cols]),
                "wo": tile_wo(Wo[cols, :]),
                "vmask": vmasks[b],
            }
        )
    return in_maps, nkt


def combine_results(results):
    y = np.zeros((B, S, D), np.float32)
    for c in range(N_CORES):
        y[c // 2] += results[c]["y"].astype(np.float32)
    return y


def kernel(x, mask, Wq, Wk, Wv, Wo):
    in_maps, nkt = make_in_maps(
        np.asarray(x, np.float32),
        np.asarray(mask),
        np.asarray(Wq, np.float32),
        np.asarray(Wk, np.float32),
        np.asarray(Wv, np.float32),
        np.asarray(Wo, np.float32),
    )
    nc = _get_nc(nkt)
    res = run_bass_kernel_spmd(nc, in_maps, core_ids=list(range(N_CORES)))
    return combine_results(res.results)
